# revision 10
# baseline (speedup 1.0000x reference)
"""CrossViewTransformer kernel for 8 Trainium2 NeuronCores (v2).

Problem: B=4, C=256, H=W=64 (N=4096), Cqk=32 cross-attention + residual.
  Q = Wq@src + bq, K = Wk@tgt + bk, V = Wv@tgt + bv   (1x1 convs)
  out = softmax(Q^T K) @ V^T + src                     (no 1/sqrt(d) scale)

Sharding: 8 cores = 4 batches x 2 query-halves. Each core: 2048 queries x
4096 keys of one batch; V/K projections over the full 4096 keys are
replicated across the 2 cores of a batch (cheap).

v2 design (per core, per iteration):
  - All matmuls fp8e4m3 at 0.5 cyc/col:
      * projections contract 256 channels as 2x128 DoubleRow partition-pairs
      * QK scores: KERNEL_QKMODE="DP" -> DoublePixel with unpaired [32,*]
        K/Q tiles (verified exact on HW); "DR" -> DoubleRow with [16,2,*]
        pair tiles
      * attn@V contracts 2 m-tiles (2x128 keys) per DoubleRow instruction
      * denominator l: ones-lhsT widened to 128 output partitions so l
        lands pre-broadcast in PSUM (no gpsimd fold/broadcast needed)
    Host pre-scales Wq/Wk/Wv (and bq/bk) by 8 to keep fp8 weights out of
    the e4m3 subnormal range; the exp activation scale (1/64) and the
    ones8 L-matmul constant (+recip) undo the scaling exactly.
  - exp split across two engines (per-group schedule, KERNEL_EXPA):
      'A': ACT table exp (exact), f32 PSUM -> fp8 SBUF
      'D': DVE Schraudolph: one fused tensor_scalar (s*A'+B) -> uint8
           whose bit pattern IS fp8e4m3(exp(s)); HW convert is
           round-to-nearest with saturation (verified)
  - software-pipelined attention: QK(g+1) is emitted before AV(g)/L(g) so
    the in-order PE queue never stalls behind an exp wait
  - normalize muls + recip on DVE reading PSUM directly; residual adds on
    the Pool engine (SBUF-only; GPSIMD cannot access PSUM); bv folded
    into the residual on host
"""

import os
import sys

sys.path.insert(0, "/opt/trn_rl_repo")

import numpy as np
import ml_dtypes

BF16 = ml_dtypes.bfloat16
FP8 = ml_dtypes.float8_e4m3

B, C, H, W = 4, 256, 64, 64
N = H * W            # 4096 keys (and queries per batch)
CQK = 32
NCORES = 8
QSH = N // 2         # 2048 queries per core
QC = 512             # q-chunk width (one PSUM bank)
NQC = QSH // QC      # 4 q-chunks
MT = 128             # m-tile (keys per scoresT tile)
NMT = N // MT        # 32 m-tiles
MG = 2               # m-tiles per group (DoubleRow pair)
NG = NMT // MG       # 16 groups per q-chunk
WSCALE = 8.0         # host pre-scale on Wq/Wk/Wv (and bq/bk)
SSCALE = 1.0 / (WSCALE * WSCALE)  # exp() input scale undoing Q*K scaling

LOOP = int(os.environ.get("KERNEL_LOOP", "0"))  # >0: repeat body for timing
# timing bisection: 0=empty loop body, 1=+proj, 2=+QK, 3=+exp, 4=+AV,
# 5=+L matmuls, 6=full
STAGE = int(os.environ.get("KERNEL_STAGE", "6"))
# exp engine split: EXPA of the 16 groups per chunk go to ACT (table exp),
# the rest to DVE (Schraudolph); spread evenly. KERNEL_EXPPAT overrides.
EXPA = int(os.environ.get("KERNEL_EXPA", "11"))
EXPPAT = os.environ.get(
    "KERNEL_EXPPAT",
    "".join("A" if (g * EXPA) % 16 + EXPA >= 16 else "D" for g in range(16)),
)
# Schraudolph bit-domain constant (HW f32->u8 convert rounds to nearest
# with saturation; C=56 centers the error, softmax cancels the +4% bias)
SCHC = float(os.environ.get("KERNEL_SCHC", "56.0"))
LOG2E = 1.4426950408889634
# V-proj PSUM->SBUF fp8 converts: VA of 16 batches on ACT, rest on DVE
VA = int(os.environ.get("KERNEL_VA", "0"))
# QK matmul mode: "DP" DoublePixel unpaired / "DR" DoubleRow cqk-pairs
QKMODE = os.environ.get("KERNEL_QKMODE", "DP")

_last_results = None  # BassKernelResults of the most recent run (for test.py)


def _build_bass():
    import concourse.bass as bass  # noqa: F401
    import concourse.tile as tile
    from concourse import bacc, mybir
    from contextlib import ExitStack

    f32 = mybir.dt.float32
    fp8 = mybir.dt.float8e4
    u8 = mybir.dt.uint8
    DR = mybir.MatmulPerfMode.DoubleRow
    DP = mybir.MatmulPerfMode.DoublePixel

    nc = bacc.Bacc("TRN2")

    # ---- DRAM I/O (per-core) ----
    tgtp_d = nc.dram_tensor("tgtp", [C, N], fp8, kind="ExternalInput")
    srcqp_d = nc.dram_tensor("srcqp", [C, QSH], fp8, kind="ExternalInput")
    srcr_d = nc.dram_tensor("srcr", [C, QSH], f32, kind="ExternalInput")
    wv_d = nc.dram_tensor("wv", [C, C], fp8, kind="ExternalInput")
    wqk_d = nc.dram_tensor("wqk", [C, 2 * CQK], fp8, kind="ExternalInput")
    bq_d = nc.dram_tensor("bq", [CQK, QC], f32, kind="ExternalInput")
    bk_d = nc.dram_tensor("bk", [CQK, 2 * MT], f32, kind="ExternalInput")
    out_d = nc.dram_tensor("out", [C, QSH], f32, kind="ExternalOutput")

    ones8_d = nc.inline_tensor(
        np.full((128, 2 * MT), WSCALE, dtype=FP8), name="ones8"
    )

    with tile.TileContext(nc) as tc:
        with (
            tc.tile_pool(name="const", bufs=1) as const,
            tc.tile_pool(name="data", bufs=1) as data,
        ):
            # ---- ACT table warmup: a dependency-free Exp so the inserted
            # ACT_TABLE_LOAD lands outside the timed loop.
            warm = const.tile([1, 8], f32, tag="warm")
            nc.vector.memset(warm, 0.0)
            nc.scalar.activation(
                out=warm, in_=warm, func=mybir.ActivationFunctionType.Exp
            )

            # ---- constants / weights ----
            wv_sb = const.tile([128, 2, C], fp8, tag="wv")
            wqk_sb = const.tile([128, 2, 2 * CQK], fp8, tag="wqk")
            ones8 = const.tile([128, 2, MT], fp8, tag="ones8")
            for j in range(2):
                nc.sync.dma_start(out=wv_sb[:, j, :], in_=wv_d[128 * j : 128 * (j + 1), :])
                nc.sync.dma_start(out=wqk_sb[:, j, :], in_=wqk_d[128 * j : 128 * (j + 1), :])
            nc.sync.dma_start(
                out=ones8, in_=ones8_d.rearrange("p (a m) -> p a m", a=2)
            )
            if QKMODE == "DP":
                bq32 = const.tile([CQK, QC], f32, tag="bq32")
                bk32 = const.tile([CQK, 2, MT], f32, tag="bk32")
                nc.sync.dma_start(out=bq32, in_=bq_d[:, :])
                nc.sync.dma_start(
                    out=bk32, in_=bk_d.rearrange("p (t m) -> p t m", t=2)
                )
            else:
                bqp = const.tile([16, 2, QC], f32, tag="bqp")
                bkp = const.tile([16, 2, 2, MT], f32, tag="bkp")
                for j in range(2):
                    nc.sync.dma_start(
                        out=bqp[:, j, :], in_=bq_d[16 * j : 16 * (j + 1), :]
                    )
                    nc.sync.dma_start(
                        out=bkp[:, j, :, :],
                        in_=bk_d[16 * j : 16 * (j + 1), :].rearrange(
                            "p (t m) -> p t m", t=2
                        ),
                    )

            # ---- big data tiles ----
            # tgt in fp8 channel-pair layout: [p, j, blk, col] = tgt[128j+p, .]
            tgtp = data.tile([128, 2, 8, QC], fp8, tag="tgtp")
            for j in range(2):
                for blk in range(8):
                    sl = slice(blk * QC, (blk + 1) * QC)
                    nc.sync.dma_start(
                        out=tgtp[:, j, blk, :], in_=tgtp_d[128 * j : 128 * (j + 1), sl]
                    )
            srcqp = data.tile([128, 2, NQC, QC], fp8, tag="srcqp")
            srcr = data.tile([128, 2, NQC, QC], f32, tag="srcr")
            for j in range(2):
                for qc in range(NQC):
                    sl = slice(qc * QC, (qc + 1) * QC)
                    nc.sync.dma_start(
                        out=srcqp[:, j, qc, :], in_=srcqp_d[128 * j : 128 * (j + 1), sl]
                    )
                    nc.sync.dma_start(
                        out=srcr[:, j, qc, :], in_=srcr_d[128 * j : 128 * (j + 1), sl]
                    )

            # projection results
            if QKMODE == "DP":
                Kp_sb = data.tile([CQK, NMT, MT], fp8, tag="Kp")
                Qp_sb = data.tile([CQK, NQC, QC], fp8, tag="Qp")
            else:
                Kp_sb = data.tile([16, 2, NMT, MT], fp8, tag="Kp")
                Qp_sb = data.tile([16, 2, NQC, QC], fp8, tag="Qp")
            VT_sb = data.tile([128, NMT, C], fp8, tag="VT")

            body_stack = ExitStack()
            if LOOP:
                body_stack.enter_context(tc.For_i(0, LOOP, 1))
            with body_stack:
                if STAGE == 0:
                    tick = data.tile([1, 8], f32, tag="tick")
                    nc.vector.memset(tick, 1.0)

                # ---- projections ----
                if STAGE >= 1:
                    with (
                        tc.tile_pool(name="pv", bufs=2, space="PSUM") as pv,
                        tc.tile_pool(name="pk", bufs=2, space="PSUM") as pk,
                        tc.tile_pool(name="pq", bufs=1, space="PSUM") as pq,
                    ):
                        def emit_q(qc):
                            if QKMODE == "DP":
                                ps = pq.tile([CQK, QC], f32, tag="psq")
                                nc.tensor.matmul(
                                    ps,
                                    lhsT=wqk_sb[:, :, 0:CQK],
                                    rhs=srcqp[:, :, qc, :],
                                    start=True, stop=True, perf_mode=DR,
                                )
                                nc.vector.tensor_add(Qp_sb[:, qc, :], ps, bq32)
                            else:
                                ps = pq.tile([16, 2, QC], f32, tag="psq")
                                for j in range(2):
                                    nc.tensor.matmul(
                                        ps[:, j, :],
                                        lhsT=wqk_sb[:, :, 16 * j : 16 * (j + 1)],
                                        rhs=srcqp[:, :, qc, :],
                                        start=True, stop=True, perf_mode=DR,
                                    )
                                nc.vector.tensor_add(Qp_sb[:, :, qc, :], ps, bqp)

                        def emit_k(kb):
                            if QKMODE == "DP":
                                ps = pk.tile([CQK, 2, MT], f32, tag="psk")
                                for t in range(2):
                                    mt = 2 * kb + t
                                    blk, o = divmod(mt * MT, QC)
                                    nc.tensor.matmul(
                                        ps[:, t, :],
                                        lhsT=wqk_sb[:, :, CQK : 2 * CQK],
                                        rhs=tgtp[:, :, blk, o : o + MT],
                                        start=True, stop=True, perf_mode=DR,
                                    )
                                nc.vector.tensor_add(
                                    Kp_sb[:, 2 * kb : 2 * kb + 2, :], ps, bk32
                                )
                            else:
                                ps = pk.tile([16, 2, 2, MT], f32, tag="psk")
                                for t in range(2):
                                    mt = 2 * kb + t
                                    blk, o = divmod(mt * MT, QC)
                                    for j in range(2):
                                        nc.tensor.matmul(
                                            ps[:, j, t, :],
                                            lhsT=wqk_sb[:, :, 32 + 16 * j : 48 + 16 * j],
                                            rhs=tgtp[:, :, blk, o : o + MT],
                                            start=True, stop=True, perf_mode=DR,
                                        )
                                nc.vector.tensor_add(
                                    Kp_sb[:, :, 2 * kb : 2 * kb + 2, :], ps, bkp
                                )

                        def emit_v(vb):
                            ps = pv.tile([128, 2, C], f32, tag="psv")
                            for t in range(2):
                                mt = 2 * vb + t
                                blk, o = divmod(mt * MT, QC)
                                nc.tensor.matmul(
                                    ps[:, t, :],
                                    lhsT=tgtp[:, :, blk, o : o + MT],
                                    rhs=wv_sb,
                                    start=True, stop=True, perf_mode=DR,
                                )
                            sl = slice(2 * vb, 2 * vb + 2)
                            if vb < VA:
                                nc.scalar.copy(out=VT_sb[:, sl, :], in_=ps)
                            else:
                                nc.vector.tensor_copy(out=VT_sb[:, sl, :], in_=ps)

                        # order: unblock chunk 0 fast (Q0, first K/V tiles),
                        # then the rest
                        emit_q(0)
                        for b in range(4):
                            emit_k(b)
                        for b in range(4):
                            emit_v(b)
                        for b in range(4, 16):
                            emit_k(b)
                            emit_v(b)
                        for qc in range(1, NQC):
                            emit_q(qc)

                # ---- attention (software-pipelined: QK one group ahead) ----
                if STAGE >= 2:
                    with (
                        tc.tile_pool(name="ps_s", bufs=2, space="PSUM") as ps_s,
                        tc.tile_pool(name="ps_av", bufs=1, space="PSUM") as ps_av,
                        tc.tile_pool(name="ps_l", bufs=1, space="PSUM") as ps_l,
                        tc.tile_pool(name="att", bufs=4) as att,
                        tc.tile_pool(name="outp", bufs=4) as outp,
                    ):
                        def emit_qk(qc, g):
                            S = ps_s.tile([128, MG, QC], f32, tag="S")
                            for i in range(MG):
                                mt = g * MG + i
                                if QKMODE == "DP":
                                    nc.tensor.matmul(
                                        S[:, i, :],
                                        lhsT=Kp_sb[:, mt, :],
                                        rhs=Qp_sb[:, qc, :],
                                        start=True, stop=True, perf_mode=DP,
                                    )
                                else:
                                    nc.tensor.matmul(
                                        S[:, i, :],
                                        lhsT=Kp_sb[:, :, mt, :],
                                        rhs=Qp_sb[:, :, qc, :],
                                        start=True, stop=True, perf_mode=DR,
                                    )
                            return S

                        groups = [(qc, g) for qc in range(NQC) for g in range(NG)]
                        av = lrow = None
                        S_next = emit_qk(*groups[0]) if STAGE >= 2 else None
                        for idx, (qc, g) in enumerate(groups):
                            if g == 0:
                                av = ps_av.tile([128, 2, QC], f32, tag="av")
                                lrow = ps_l.tile([128, QC], f32, tag="lrow")
                            S_cur = S_next
                            if idx + 1 < len(groups):
                                S_next = emit_qk(*groups[idx + 1])
                            expT = att.tile([128, MG, QC], fp8, tag="expT")
                            if STAGE >= 3:
                                if EXPPAT[g % len(EXPPAT)] == "A":
                                    nc.scalar.activation(
                                        out=expT.rearrange("p a b -> p (a b)"),
                                        in_=S_cur.rearrange("p a b -> p (a b)"),
                                        func=mybir.ActivationFunctionType.Exp,
                                        scale=SSCALE,
                                    )
                                else:
                                    nc.vector.tensor_scalar(
                                        expT.rearrange("p a b -> p (a b)").bitcast(u8),
                                        S_cur.rearrange("p a b -> p (a b)"),
                                        8.0 * LOG2E * SSCALE,
                                        SCHC,
                                        mybir.AluOpType.mult,
                                        mybir.AluOpType.add,
                                    )
                            if STAGE >= 4:
                                mt0 = g * MG
                                for h in range(2):
                                    nc.tensor.matmul(
                                        av[:, h, :],
                                        lhsT=VT_sb[:, mt0 : mt0 + 2, 128 * h : 128 * (h + 1)],
                                        rhs=expT,
                                        start=g == 0,
                                        stop=g == NG - 1,
                                        perf_mode=DR,
                                    )
                            if STAGE >= 5:
                                # l (x WSCALE) broadcast to all 128 partitions
                                # at no extra PE cost
                                nc.tensor.matmul(
                                    lrow,
                                    lhsT=ones8,
                                    rhs=expT,
                                    start=g == 0,
                                    stop=g == NG - 1,
                                    perf_mode=DR,
                                )
                            if STAGE < 6 or g != NG - 1:
                                continue
                            # tail: r = 1/(8*l); o = av8*r + srcr
                            r_rep = outp.tile([128, QC], f32, tag="r_rep")
                            nc.vector.reciprocal_approx_fast(out=r_rep, in_=lrow)
                            for h in range(2):
                                o = outp.tile([128, QC], f32, tag=f"o{h}")
                                nc.vector.tensor_mul(o, av[:, h, :], r_rep)
                                nc.gpsimd.tensor_add(o, o, srcr[:, h, qc, :])
                                nc.sync.dma_start(
                                    out=out_d[
                                        128 * h : 128 * (h + 1),
                                        qc * QC : (qc + 1) * QC,
                                    ],
                                    in_=o,
                                )
    nc.compile()
    return nc


_cached = None


def _get_bass():
    global _cached
    if _cached is None:
        _cached = _build_bass()
    return _cached


def make_in_maps(src_feat, tgt_feat, Wq, bq, Wk, bk, Wv, bv):
    """Host-side shard + layout prep shared by kernel() and test.py."""
    src = np.asarray(src_feat, dtype=np.float32).reshape(B, C, N)
    tgt = np.asarray(tgt_feat, dtype=np.float32).reshape(B, C, N)
    # weights scaled by 8 to keep fp8 out of subnormals; wqk = [WqT8 | WkT8]
    wqkT = np.concatenate(
        [np.asarray(Wq, np.float32).T, np.asarray(Wk, np.float32).T], axis=1
    )
    wqk8 = np.ascontiguousarray(wqkT * WSCALE).astype(FP8)
    wv8 = np.ascontiguousarray(np.asarray(Wv, np.float32).T * WSCALE).astype(FP8)
    # biases broadcast along the moving dim (x8 to match weight scaling)
    bq8 = np.asarray(bq, np.float32) * WSCALE
    bk8 = np.asarray(bk, np.float32) * WSCALE
    bq_t = np.ascontiguousarray(np.tile(bq8[:, None], (1, QC)))
    bk_t = np.ascontiguousarray(np.tile(bk8[:, None], (1, 2 * MT)))

    tgt_f8 = tgt.astype(FP8)
    src_f8 = src.astype(FP8)
    srcr_full = src + np.asarray(bv, np.float32)[None, :, None]

    in_maps = []
    for c in range(NCORES):
        b, h = divmod(c, 2)
        qsl = slice(h * QSH, (h + 1) * QSH)
        in_maps.append(
            {
                "tgtp": np.ascontiguousarray(tgt_f8[b]),
                "srcqp": np.ascontiguousarray(src_f8[b, :, qsl]),
                "srcr": np.ascontiguousarray(srcr_full[b, :, qsl]),
                "wv": wv8,
                "wqk": wqk8,
                "bq": bq_t,
                "bk": bk_t,
            }
        )
    return in_maps


def kernel(src_feat, tgt_feat, Wq, bq, Wk, bk, Wv, bv):
    """Full inputs in, full output out. Shards internally across 8 cores."""
    global _last_results
    from concourse.bass_utils import run_bass_kernel_spmd

    in_maps = make_in_maps(src_feat, tgt_feat, Wq, bq, Wk, bk, Wv, bv)

    nc = _get_bass()
    res = None
    for attempt in range(3):
        try:
            res = run_bass_kernel_spmd(
                nc,
                in_maps,
                core_ids=list(range(NCORES)),
                trace=bool(int(os.environ.get("KERNEL_TRACE", "0"))),
            )
            break
        except Exception:
            # the axon-tunneled devices occasionally report
            # NRT_EXEC_UNIT_UNRECOVERABLE; a retry on a fresh execute recovers
            if attempt == 2:
                raise
            import time as _time

            _time.sleep(5)
    _last_results = res

    out = np.empty((B, C, N), dtype=np.float32)
    for c in range(NCORES):
        b, h = divmod(c, 2)
        out[b, :, h * QSH : (h + 1) * QSH] = res.results[c]["out"]
    return out.reshape(B, C, H, W)


# revision 13
# speedup vs baseline: 1.1414x; 1.1414x over previous
"""CrossViewTransformer kernel for 8 Trainium2 NeuronCores (v2).

Problem: B=4, C=256, H=W=64 (N=4096), Cqk=32 cross-attention + residual.
  Q = Wq@src + bq, K = Wk@tgt + bk, V = Wv@tgt + bv   (1x1 convs)
  out = softmax(Q^T K) @ V^T + src                     (no 1/sqrt(d) scale)

Sharding: 8 cores = 4 batches x 2 query-halves. Each core: 2048 queries x
4096 keys of one batch; V/K projections over the full 4096 keys are
replicated across the 2 cores of a batch (cheap).

v2 design (per core, per iteration):
  - All matmuls fp8e4m3 at 0.5 cyc/col:
      * projections contract 256 channels as 2x128 DoubleRow partition-pairs
      * QK scores: KERNEL_QKMODE="DP" -> DoublePixel with unpaired [32,*]
        K/Q tiles (verified exact on HW); "DR" -> DoubleRow with [16,2,*]
        pair tiles
      * attn@V contracts 2 m-tiles (2x128 keys) per DoubleRow instruction
      * denominator l: ones-lhsT widened to 128 output partitions so l
        lands pre-broadcast in PSUM (no gpsimd fold/broadcast needed)
    Host pre-scales Wq/Wk/Wv (and bq/bk) by 8 to keep fp8 weights out of
    the e4m3 subnormal range; the exp activation scale (1/64) and the
    ones8 L-matmul constant (+recip) undo the scaling exactly.
  - exp split across two engines (per-group schedule, KERNEL_EXPA):
      'A': ACT table exp (exact), f32 PSUM -> fp8 SBUF
      'D': DVE Schraudolph: one fused tensor_scalar (s*A'+B) -> uint8
           whose bit pattern IS fp8e4m3(exp(s)); HW convert is
           round-to-nearest with saturation (verified)
  - software-pipelined attention: QK(g+1) is emitted before AV(g)/L(g) so
    the in-order PE queue never stalls behind an exp wait
  - normalize muls + recip on DVE reading PSUM directly; residual adds on
    the Pool engine (SBUF-only; GPSIMD cannot access PSUM); bv folded
    into the residual on host
"""

import os
import sys

sys.path.insert(0, "/opt/trn_rl_repo")

import numpy as np
import ml_dtypes

BF16 = ml_dtypes.bfloat16
FP8 = ml_dtypes.float8_e4m3

B, C, H, W = 4, 256, 64, 64
N = H * W            # 4096 keys (and queries per batch)
CQK = 32
NCORES = 8
QSH = N // 2         # 2048 queries per core
QC = 512             # q-chunk width (one PSUM bank)
NQC = QSH // QC      # 4 q-chunks
MT = 128             # m-tile (keys per scoresT tile)
NMT = N // MT        # 32 m-tiles
MG = 2               # m-tiles per group (DoubleRow pair)
NG = NMT // MG       # 16 groups per q-chunk
WSCALE = 8.0         # host pre-scale on Wq/Wk/Wv (and bq/bk)
SSCALE = 1.0 / (WSCALE * WSCALE)  # exp() input scale undoing Q*K scaling

LOOP = int(os.environ.get("KERNEL_LOOP", "0"))  # >0: repeat body for timing
# timing bisection: 0=empty loop body, 1=+proj, 2=+QK, 3=+exp, 4=+AV,
# 5=+L matmuls, 6=full
STAGE = int(os.environ.get("KERNEL_STAGE", "6"))
# exp engine split: EXPA of the 16 groups per chunk go to ACT (table exp),
# the rest to DVE (Schraudolph); spread evenly. KERNEL_EXPPAT overrides.
EXPA = int(os.environ.get("KERNEL_EXPA", "11"))
EXPPAT = os.environ.get(
    "KERNEL_EXPPAT",
    "".join("A" if (g * EXPA) % 16 + EXPA >= 16 else "D" for g in range(16)),
)
# Schraudolph bit-domain constant (HW f32->u8 convert rounds to nearest
# with saturation; C=56 centers the error, softmax cancels the +4% bias)
SCHC = float(os.environ.get("KERNEL_SCHC", "56.0"))
LOG2E = 1.4426950408889634
# V-proj PSUM->SBUF fp8 converts: VA of 16 batches on ACT, rest on DVE
VA = int(os.environ.get("KERNEL_VA", "0"))
# QK matmul mode: "DP" DoublePixel unpaired / "DR" DoubleRow cqk-pairs
QKMODE = os.environ.get("KERNEL_QKMODE", "DP")

_last_results = None  # BassKernelResults of the most recent run (for test.py)


def _build_bass():
    import concourse.bass as bass  # noqa: F401
    import concourse.tile as tile
    from concourse import bacc, mybir
    from contextlib import ExitStack

    f32 = mybir.dt.float32
    fp8 = mybir.dt.float8e4
    u8 = mybir.dt.uint8
    DR = mybir.MatmulPerfMode.DoubleRow
    DP = mybir.MatmulPerfMode.DoublePixel

    nc = bacc.Bacc("TRN2")

    # ---- DRAM I/O (per-core) ----
    tgtp_d = nc.dram_tensor("tgtp", [C, N], fp8, kind="ExternalInput")
    srcqp_d = nc.dram_tensor("srcqp", [C, QSH], fp8, kind="ExternalInput")
    srcr_d = nc.dram_tensor("srcr", [C, QSH], f32, kind="ExternalInput")
    wv_d = nc.dram_tensor("wv", [C, C], fp8, kind="ExternalInput")
    wqk_d = nc.dram_tensor("wqk", [C, 2 * CQK], fp8, kind="ExternalInput")
    bq_d = nc.dram_tensor("bq", [CQK, QC], f32, kind="ExternalInput")
    bk_d = nc.dram_tensor("bk", [CQK, 2 * MT], f32, kind="ExternalInput")
    out_d = nc.dram_tensor("out", [C, QSH], f32, kind="ExternalOutput")

    ones8_d = nc.inline_tensor(
        np.full((128, 2 * MT), WSCALE, dtype=FP8), name="ones8"
    )

    with tile.TileContext(nc) as tc:
        with (
            tc.tile_pool(name="const", bufs=1) as const,
            tc.tile_pool(name="data", bufs=1) as data,
        ):
            # ---- ACT table warmup: a dependency-free Exp so the inserted
            # ACT_TABLE_LOAD lands outside the timed loop.
            warm = const.tile([1, 8], f32, tag="warm")
            nc.vector.memset(warm, 0.0)
            nc.scalar.activation(
                out=warm, in_=warm, func=mybir.ActivationFunctionType.Exp
            )

            # ---- constants / weights ----
            wv_sb = const.tile([128, 2, C], fp8, tag="wv")
            wqk_sb = const.tile([128, 2, 2 * CQK], fp8, tag="wqk")
            ones8 = const.tile([128, 2, MT], fp8, tag="ones8")
            for j in range(2):
                nc.sync.dma_start(out=wv_sb[:, j, :], in_=wv_d[128 * j : 128 * (j + 1), :])
                nc.sync.dma_start(out=wqk_sb[:, j, :], in_=wqk_d[128 * j : 128 * (j + 1), :])
            nc.sync.dma_start(
                out=ones8, in_=ones8_d.rearrange("p (a m) -> p a m", a=2)
            )
            if QKMODE == "DP":
                bq32 = const.tile([CQK, QC], f32, tag="bq32")
                bk32 = const.tile([CQK, 2, MT], f32, tag="bk32")
                nc.sync.dma_start(out=bq32, in_=bq_d[:, :])
                nc.sync.dma_start(
                    out=bk32, in_=bk_d.rearrange("p (t m) -> p t m", t=2)
                )
            else:
                bqp = const.tile([16, 2, QC], f32, tag="bqp")
                bkp = const.tile([16, 2, 2, MT], f32, tag="bkp")
                for j in range(2):
                    nc.sync.dma_start(
                        out=bqp[:, j, :], in_=bq_d[16 * j : 16 * (j + 1), :]
                    )
                    nc.sync.dma_start(
                        out=bkp[:, j, :, :],
                        in_=bk_d[16 * j : 16 * (j + 1), :].rearrange(
                            "p (t m) -> p t m", t=2
                        ),
                    )

            # ---- big data tiles ----
            # tgt in fp8 channel-pair layout: [p, j, blk, col] = tgt[128j+p, .]
            tgtp = data.tile([128, 2, 8, QC], fp8, tag="tgtp")
            for j in range(2):
                for blk in range(8):
                    sl = slice(blk * QC, (blk + 1) * QC)
                    nc.sync.dma_start(
                        out=tgtp[:, j, blk, :], in_=tgtp_d[128 * j : 128 * (j + 1), sl]
                    )
            srcqp = data.tile([128, 2, NQC, QC], fp8, tag="srcqp")
            srcr = data.tile([128, 2, NQC, QC], f32, tag="srcr")
            for j in range(2):
                for qc in range(NQC):
                    sl = slice(qc * QC, (qc + 1) * QC)
                    nc.sync.dma_start(
                        out=srcqp[:, j, qc, :], in_=srcqp_d[128 * j : 128 * (j + 1), sl]
                    )
                    nc.sync.dma_start(
                        out=srcr[:, j, qc, :], in_=srcr_d[128 * j : 128 * (j + 1), sl]
                    )

            # projection results
            if QKMODE == "DP":
                Kp_sb = data.tile([CQK, NMT, MT], fp8, tag="Kp")
                Qp_sb = data.tile([CQK, NQC, QC], fp8, tag="Qp")
            else:
                Kp_sb = data.tile([16, 2, NMT, MT], fp8, tag="Kp")
                Qp_sb = data.tile([16, 2, NQC, QC], fp8, tag="Qp")
            VT_sb = data.tile([128, NMT, C], fp8, tag="VT")

            body_stack = ExitStack()
            if LOOP:
                body_stack.enter_context(tc.For_i(0, LOOP, 1))
            with body_stack:
                if STAGE == 0:
                    tick = data.tile([1, 8], f32, tag="tick")
                    nc.vector.memset(tick, 1.0)

                # ---- projections ----
                if STAGE >= 1:
                    with (
                        tc.tile_pool(name="pv", bufs=2, space="PSUM") as pv,
                        tc.tile_pool(name="pk", bufs=2, space="PSUM") as pk,
                        tc.tile_pool(name="pq", bufs=1, space="PSUM") as pq,
                    ):
                        def emit_q(qc):
                            if QKMODE == "DP":
                                ps = pq.tile([CQK, QC], f32, tag="psq")
                                nc.tensor.matmul(
                                    ps,
                                    lhsT=wqk_sb[:, :, 0:CQK],
                                    rhs=srcqp[:, :, qc, :],
                                    start=True, stop=True, perf_mode=DR,
                                )
                                nc.vector.tensor_add(Qp_sb[:, qc, :], ps, bq32)
                            else:
                                ps = pq.tile([16, 2, QC], f32, tag="psq")
                                for j in range(2):
                                    nc.tensor.matmul(
                                        ps[:, j, :],
                                        lhsT=wqk_sb[:, :, 16 * j : 16 * (j + 1)],
                                        rhs=srcqp[:, :, qc, :],
                                        start=True, stop=True, perf_mode=DR,
                                    )
                                nc.vector.tensor_add(Qp_sb[:, :, qc, :], ps, bqp)

                        def emit_k(kb):
                            if QKMODE == "DP":
                                ps = pk.tile([CQK, 2, MT], f32, tag="psk")
                                for t in range(2):
                                    mt = 2 * kb + t
                                    blk, o = divmod(mt * MT, QC)
                                    nc.tensor.matmul(
                                        ps[:, t, :],
                                        lhsT=wqk_sb[:, :, CQK : 2 * CQK],
                                        rhs=tgtp[:, :, blk, o : o + MT],
                                        start=True, stop=True, perf_mode=DR,
                                    )
                                nc.vector.tensor_add(
                                    Kp_sb[:, 2 * kb : 2 * kb + 2, :], ps, bk32
                                )
                            else:
                                ps = pk.tile([16, 2, 2, MT], f32, tag="psk")
                                for t in range(2):
                                    mt = 2 * kb + t
                                    blk, o = divmod(mt * MT, QC)
                                    for j in range(2):
                                        nc.tensor.matmul(
                                            ps[:, j, t, :],
                                            lhsT=wqk_sb[:, :, 32 + 16 * j : 48 + 16 * j],
                                            rhs=tgtp[:, :, blk, o : o + MT],
                                            start=True, stop=True, perf_mode=DR,
                                        )
                                nc.vector.tensor_add(
                                    Kp_sb[:, :, 2 * kb : 2 * kb + 2, :], ps, bkp
                                )

                        def emit_v(vb):
                            ps = pv.tile([128, 2, C], f32, tag="psv")
                            for t in range(2):
                                mt = 2 * vb + t
                                blk, o = divmod(mt * MT, QC)
                                nc.tensor.matmul(
                                    ps[:, t, :],
                                    lhsT=tgtp[:, :, blk, o : o + MT],
                                    rhs=wv_sb,
                                    start=True, stop=True, perf_mode=DR,
                                )
                            sl = slice(2 * vb, 2 * vb + 2)
                            if vb < VA:
                                nc.scalar.copy(out=VT_sb[:, sl, :], in_=ps)
                            else:
                                nc.vector.tensor_copy(out=VT_sb[:, sl, :], in_=ps)

                        # order: unblock chunk 0 fast (Q0, first K/V tiles),
                        # then the rest
                        emit_q(0)
                        for b in range(4):
                            emit_k(b)
                        for b in range(4):
                            emit_v(b)
                        for b in range(4, 16):
                            emit_k(b)
                            emit_v(b)
                        for qc in range(1, NQC):
                            emit_q(qc)

                # ---- attention (software-pipelined: QK one group ahead) ----
                if STAGE >= 2:
                    with (
                        tc.tile_pool(name="ps_s", bufs=2, space="PSUM") as ps_s,
                        tc.tile_pool(name="ps_av", bufs=1, space="PSUM") as ps_av,
                        tc.tile_pool(name="ps_l", bufs=1, space="PSUM") as ps_l,
                        tc.tile_pool(name="att", bufs=4) as att,
                        tc.tile_pool(name="outp", bufs=4) as outp,
                    ):
                        def emit_qk(qc, g):
                            S = ps_s.tile([128, MG, QC], f32, tag="S")
                            for i in range(MG):
                                mt = g * MG + i
                                if QKMODE == "DP":
                                    nc.tensor.matmul(
                                        S[:, i, :],
                                        lhsT=Kp_sb[:, mt, :],
                                        rhs=Qp_sb[:, qc, :],
                                        start=True, stop=True, perf_mode=DP,
                                    )
                                else:
                                    nc.tensor.matmul(
                                        S[:, i, :],
                                        lhsT=Kp_sb[:, :, mt, :],
                                        rhs=Qp_sb[:, :, qc, :],
                                        start=True, stop=True, perf_mode=DR,
                                    )
                            return S

                        def make_tail(qc, av, lrow):
                            # tail: r = 1/(8*l); o = av8*r + srcr. Emitted
                            # DELAYED (after the next chunk's first exp) so
                            # the recip's wait on L(NG-1) doesn't block the
                            # in-order DVE queue; must land before the next
                            # chunk's first AV (av/lrow bank reuse).
                            def tail():
                                r_rep = outp.tile([128, QC], f32, tag="r_rep")
                                nc.vector.reciprocal_approx_fast(
                                    out=r_rep, in_=lrow
                                )
                                for h in range(2):
                                    o = outp.tile([128, QC], f32, tag=f"o{h}")
                                    nc.vector.tensor_mul(o, av[:, h, :], r_rep)
                                    nc.gpsimd.tensor_add(o, o, srcr[:, h, qc, :])
                                    nc.sync.dma_start(
                                        out=out_d[
                                            128 * h : 128 * (h + 1),
                                            qc * QC : (qc + 1) * QC,
                                        ],
                                        in_=o,
                                    )
                            return tail

                        groups = [(qc, g) for qc in range(NQC) for g in range(NG)]
                        av = lrow = None
                        pending_tail = None
                        S_next = emit_qk(*groups[0]) if STAGE >= 2 else None
                        for idx, (qc, g) in enumerate(groups):
                            if g == 0:
                                av = ps_av.tile([128, 2, QC], f32, tag="av")
                                lrow = ps_l.tile([128, QC], f32, tag="lrow")
                            S_cur = S_next
                            if idx + 1 < len(groups):
                                S_next = emit_qk(*groups[idx + 1])
                            expT = att.tile([128, MG, QC], fp8, tag="expT")
                            if STAGE >= 3:
                                if EXPPAT[g % len(EXPPAT)] == "A":
                                    nc.scalar.activation(
                                        out=expT.rearrange("p a b -> p (a b)"),
                                        in_=S_cur.rearrange("p a b -> p (a b)"),
                                        func=mybir.ActivationFunctionType.Exp,
                                        scale=SSCALE,
                                    )
                                else:
                                    nc.vector.tensor_scalar(
                                        expT.rearrange("p a b -> p (a b)").bitcast(u8),
                                        S_cur.rearrange("p a b -> p (a b)"),
                                        8.0 * LOG2E * SSCALE,
                                        SCHC,
                                        mybir.AluOpType.mult,
                                        mybir.AluOpType.add,
                                    )
                            if pending_tail is not None:
                                pending_tail()
                                pending_tail = None
                            if STAGE >= 4:
                                mt0 = g * MG
                                for h in range(2):
                                    nc.tensor.matmul(
                                        av[:, h, :],
                                        lhsT=VT_sb[:, mt0 : mt0 + 2, 128 * h : 128 * (h + 1)],
                                        rhs=expT,
                                        start=g == 0,
                                        stop=g == NG - 1,
                                        perf_mode=DR,
                                    )
                            if STAGE >= 5:
                                # l (x WSCALE) broadcast to all 128 partitions
                                # at no extra PE cost
                                nc.tensor.matmul(
                                    lrow,
                                    lhsT=ones8,
                                    rhs=expT,
                                    start=g == 0,
                                    stop=g == NG - 1,
                                    perf_mode=DR,
                                )
                            if STAGE < 6 or g != NG - 1:
                                continue
                            pending_tail = make_tail(qc, av, lrow)
                        if pending_tail is not None:
                            pending_tail()
    nc.compile()
    return nc


_cached = None


def _get_bass():
    global _cached
    if _cached is None:
        _cached = _build_bass()
    return _cached


def make_in_maps(src_feat, tgt_feat, Wq, bq, Wk, bk, Wv, bv):
    """Host-side shard + layout prep shared by kernel() and test.py."""
    src = np.asarray(src_feat, dtype=np.float32).reshape(B, C, N)
    tgt = np.asarray(tgt_feat, dtype=np.float32).reshape(B, C, N)
    # weights scaled by 8 to keep fp8 out of subnormals; wqk = [WqT8 | WkT8]
    wqkT = np.concatenate(
        [np.asarray(Wq, np.float32).T, np.asarray(Wk, np.float32).T], axis=1
    )
    wqk8 = np.ascontiguousarray(wqkT * WSCALE).astype(FP8)
    wv8 = np.ascontiguousarray(np.asarray(Wv, np.float32).T * WSCALE).astype(FP8)
    # biases broadcast along the moving dim (x8 to match weight scaling)
    bq8 = np.asarray(bq, np.float32) * WSCALE
    bk8 = np.asarray(bk, np.float32) * WSCALE
    bq_t = np.ascontiguousarray(np.tile(bq8[:, None], (1, QC)))
    bk_t = np.ascontiguousarray(np.tile(bk8[:, None], (1, 2 * MT)))

    tgt_f8 = tgt.astype(FP8)
    src_f8 = src.astype(FP8)
    srcr_full = src + np.asarray(bv, np.float32)[None, :, None]

    in_maps = []
    for c in range(NCORES):
        b, h = divmod(c, 2)
        qsl = slice(h * QSH, (h + 1) * QSH)
        in_maps.append(
            {
                "tgtp": np.ascontiguousarray(tgt_f8[b]),
                "srcqp": np.ascontiguousarray(src_f8[b, :, qsl]),
                "srcr": np.ascontiguousarray(srcr_full[b, :, qsl]),
                "wv": wv8,
                "wqk": wqk8,
                "bq": bq_t,
                "bk": bk_t,
            }
        )
    return in_maps


def kernel(src_feat, tgt_feat, Wq, bq, Wk, bk, Wv, bv):
    """Full inputs in, full output out. Shards internally across 8 cores."""
    global _last_results
    from concourse.bass_utils import run_bass_kernel_spmd

    in_maps = make_in_maps(src_feat, tgt_feat, Wq, bq, Wk, bk, Wv, bv)

    nc = _get_bass()
    res = None
    for attempt in range(3):
        try:
            res = run_bass_kernel_spmd(
                nc,
                in_maps,
                core_ids=list(range(NCORES)),
                trace=bool(int(os.environ.get("KERNEL_TRACE", "0"))),
            )
            break
        except Exception:
            # the axon-tunneled devices occasionally report
            # NRT_EXEC_UNIT_UNRECOVERABLE; a retry on a fresh execute recovers
            if attempt == 2:
                raise
            import time as _time

            _time.sleep(5)
    _last_results = res

    out = np.empty((B, C, N), dtype=np.float32)
    for c in range(NCORES):
        b, h = divmod(c, 2)
        out[b, :, h * QSH : (h + 1) * QSH] = res.results[c]["out"]
    return out.reshape(B, C, H, W)
